# revision 10
# baseline (speedup 1.0000x reference)
"""3-layer GAT on Trainium2, 8 NeuronCores, Bass/Tile.

Strategy (graph-parallel per sharding hint):
  - Launch A: row-sharded fp32 matmul H = x @ W1_aug (aug columns carry the
    per-head attention scalars a_src/a_dst folded into the weights).
  - Host: assemble a gather table [50000, 2112] fp32 whose rows interleave a
    ones-column per head ([h feats(256) | 1.0] x 8 | a_src(8) | a_dst(8) | pad)
    so softmax numerator and denominator come out of one PSUM accumulation.
  - Launch B: destination-sharded layer-1 aggregation. Edges sorted by dst,
    128-edge chunks; per chunk one dma_gather (8448B rows), one fused DVE op
    builds all 8 one-hot*exp scatter matrices, 8 fp32 matmuls accumulate
    numerator+denominator in PSUM. Finalize: divide, +b1, ELU -> emb shard;
    then z23_aug = emb @ [W2|W3]_aug per-row shard.
  - Launch C: layers 2/3 aggregation on z-tables (768B rows), feature softmax,
    argmax -> per-core logits/predictions shards. Host concatenates.

All matmuls fp32 (4 cyc/row) to keep the argmax `predictions` output faithful
to the fp32 reference.
"""
import os
import sys
import time

sys.path.insert(0, "/opt/trn_rl_repo")
sys.path.insert(0, "/opt/pypackages")

import numpy as np

import concourse.bacc as bacc
import concourse.mybir as mybir
import concourse.tile as tile
from concourse.bass import ds
import jax
from jax.sharding import Mesh, PartitionSpec, NamedSharding
from jax.experimental.shard_map import shard_map
from concourse.bass2jax import _bass_exec_p, install_neuronx_cc_hook, partition_id_tensor

P = 128
NCORES = 8
AF = mybir.ActivationFunctionType
ALU = mybir.AluOpType
F32 = mybir.dt.float32
I16 = mybir.dt.int16
I32 = mybir.dt.int32
NEG_SLOPE = 0.2

VERBOSE = os.environ.get("KERNEL_VERBOSE", "1") == "1"
BACKEND = os.environ.get("KERNEL_BACKEND", "hw")  # hw | sim


def _log(*a):
    if VERBOSE:
        print("[kernel]", *a, flush=True)


class SimRunner:
    """CoreSim-based runner (no hardware): runs each core's program in the
    interpreter. Slow; for debugging only."""

    def __init__(self, nc, shared_names, n_cores=NCORES):
        from concourse.bass_interp import CoreSim
        self.nc = nc
        self.n_cores = n_cores
        self.shared = set(shared_names)
        self.CoreSim = CoreSim
        self.out_names = []
        self.in_names = []
        for alloc in nc.m.functions[0].allocations:
            if not isinstance(alloc, mybir.MemoryLocationSet):
                continue
            name = alloc.memorylocations[0].name
            if alloc.kind == "ExternalInput":
                self.in_names.append(name)
            elif alloc.kind == "ExternalOutput":
                self.out_names.append(name)

    def put_inputs(self, shared_map, per_core_maps):
        self._shared = shared_map
        self._per_core = per_core_maps

    def run(self):
        res = []
        for c in range(self.n_cores):
            sim = self.CoreSim(self.nc, require_finite=False, require_nnan=False)
            for nm in self.in_names:
                src = self._shared.get(nm) if nm in self.shared else \
                    self._per_core[c].get(nm)
                if src is None:
                    continue
                sim.tensor(nm)[:] = src
            sim.simulate(check_with_hw=False)
            res.append({nm: np.array(sim.tensor(nm)) for nm in self.out_names})
            _log(f"  sim core {c} done")
        return res

    def results(self, outs):
        return outs

    def time_runs(self, n):
        return [0.0]


def make_runner(nc, shared_names):
    if BACKEND == "sim":
        return SimRunner(nc, shared_names)
    return SpmdRunner(nc, shared_names)


# ----------------------------------------------------------------------------
# SPMD runner (device-resident inputs, shared vs per-core sharding)
# ----------------------------------------------------------------------------
class SpmdRunner:
    def __init__(self, nc, shared_names, n_cores=NCORES):
        install_neuronx_cc_hook()
        self.nc = nc
        self.n_cores = n_cores
        partition_name = nc.partition_id_tensor.name if nc.partition_id_tensor else None
        in_names, out_names, out_avals, zero_outs = [], [], [], []
        for alloc in nc.m.functions[0].allocations:
            if not isinstance(alloc, mybir.MemoryLocationSet):
                continue
            name = alloc.memorylocations[0].name
            if alloc.kind == "ExternalInput":
                if name != partition_name:
                    in_names.append(name)
            elif alloc.kind == "ExternalOutput":
                out_names.append(name)
                shape = tuple(alloc.tensor_shape)
                dtype = mybir.dt.np(alloc.dtype)
                out_avals.append(jax.core.ShapedArray(shape, dtype))
                zero_outs.append(np.zeros(shape, dtype))
        self.in_names = in_names
        self.shared = set(shared_names)
        self.out_names = out_names
        self.out_avals = out_avals
        self.zero_outs = zero_outs
        n_outs = len(out_names)
        all_in_names = in_names + out_names
        if partition_name is not None:
            all_in_names.append(partition_name)

        def _body(*args):
            operands = list(args)
            if partition_name is not None:
                operands.append(partition_id_tensor())
            outs = _bass_exec_p.bind(
                *operands,
                out_avals=tuple(out_avals),
                in_names=tuple(all_in_names),
                out_names=tuple(out_names),
                lowering_input_output_aliases=(),
                sim_require_finite=True,
                sim_require_nnan=True,
                nc=nc,
            )
            return tuple(outs)

        devices = jax.devices()[:n_cores]
        self.mesh = Mesh(np.asarray(devices), ("core",))
        in_specs = tuple(
            PartitionSpec() if nm in self.shared else PartitionSpec("core")
            for nm in in_names
        ) + (PartitionSpec("core"),) * n_outs
        out_specs = (PartitionSpec("core"),) * n_outs
        self.shard_spec = NamedSharding(self.mesh, PartitionSpec("core"))
        self.repl_spec = NamedSharding(self.mesh, PartitionSpec())
        self.jitted = jax.jit(
            shard_map(_body, mesh=self.mesh, in_specs=in_specs,
                      out_specs=out_specs, check_rep=False),
            keep_unused=True,
        )

    def put_inputs(self, shared_map, per_core_maps):
        dev_in = []
        for nm in self.in_names:
            if nm in self.shared:
                a = np.ascontiguousarray(shared_map[nm])
                dev_in.append(jax.device_put(a, self.repl_spec))
            else:
                a = np.concatenate(
                    [np.ascontiguousarray(m[nm]) for m in per_core_maps], axis=0)
                dev_in.append(jax.device_put(a, self.shard_spec))
        self._dev_in = dev_in
        self._dev_zero = [
            jax.device_put(
                np.zeros((self.n_cores * z.shape[0], *z.shape[1:]), z.dtype),
                self.shard_spec)
            for z in self.zero_outs
        ]
        for a in self._dev_in + self._dev_zero:
            a.block_until_ready()

    def run(self):
        outs = self.jitted(*self._dev_in, *self._dev_zero)
        for o in outs:
            o.block_until_ready()
        return outs

    def time_runs(self, n):
        ts = []
        for _ in range(n):
            t0 = time.perf_counter()
            outs = self.jitted(*self._dev_in, *self._dev_zero)
            for o in outs:
                o.block_until_ready()
            ts.append(time.perf_counter() - t0)
        return ts

    def results(self, outs):
        res = []
        np_outs = [np.asarray(o) for o in outs]
        for c in range(self.n_cores):
            d = {}
            for i, nm in enumerate(self.out_names):
                shp = self.out_avals[i].shape
                d[nm] = np_outs[i].reshape(self.n_cores, *shp)[c]
            res.append(d)
        return res


# ----------------------------------------------------------------------------
# Host-side graph sharding
# ----------------------------------------------------------------------------
def prep_graph_shard(src, dst, n, core, nper, half, ntiles_core):
    """Edges with dst in this core's range, sorted by dst, tiled by 128 dsts,
    each tile's edges split lo/hi by src < half, each group padded to x128.

    Returns dict with wrapped idx arrays and chunk schedule.
    """
    lo = core * nper
    hi = lo + nper
    m = (dst >= lo) & (dst < hi)
    es = src[m]
    ed = dst[m] - lo
    order = np.argsort(ed, kind="stable")
    es, ed = es[order], ed[order]

    src_idx_parts, adst_idx_parts, dstloc_parts = [], [], []
    sched = []  # list over tiles: (nchunks_lo, nchunks_hi)
    tile_of = ed // P
    tile_starts = np.searchsorted(tile_of, np.arange(ntiles_core + 1))
    for t in range(ntiles_core):
        s, e = tile_starts[t], tile_starts[t + 1]
        ts_, td_ = es[s:e], ed[s:e]
        is_lo = ts_ < half
        nch = []
        for sel, base in ((is_lo, 0), (~is_lo, half)):
            gs, gd = ts_[sel], td_[sel]
            n_e = gs.shape[0]
            n_c = -(-n_e // P) if n_e else 0
            pad = n_c * P - n_e
            # pad edges: src row 0 of this half, adst pad row (-1e9), dst_local 0
            sidx = np.concatenate([gs - base, np.zeros(pad, np.int64)])
            aidx = np.concatenate([gd, np.full(pad, nper, np.int64)])
            dloc = np.concatenate([gd - t * P, np.zeros(pad, np.int64)])
            src_idx_parts.append(sidx)
            adst_idx_parts.append(aidx)
            dstloc_parts.append(dloc)
            nch.append(n_c)
        sched.append(tuple(nch))

    src_idx = np.concatenate(src_idx_parts).astype(np.int16)
    adst_idx = np.concatenate(adst_idx_parts).astype(np.int16)
    dst_loc = np.concatenate(dstloc_parts).astype(np.float32)
    epad = src_idx.shape[0]
    assert epad % P == 0
    # combined wrapped idx rows: [128, W] with src idx cols then adst idx cols
    # src idx j at [j%16, j//16] replicated to 8 groups of 16 partitions
    w16 = epad // 16
    srcw = np.tile(src_idx.reshape(-1, 16).T, (8, 1))      # [128, w16]
    adstw = np.tile(adst_idx.reshape(-1, 16).T, (8, 1))    # [128, w16]
    dstlocw = dst_loc.reshape(-1, P).T.copy()              # [128, epad/128]
    return dict(srcw=srcw, adstw=adstw, dstlocw=dstlocw, sched=sched,
                nchunks=epad // P, w16=w16)


def make_gather_groups(sched, cpg):
    """Yield (tile, chunk_base, group_size, half, first_of_tile, last_of_tile).

    chunk_base indexes the flat chunk stream. Groups never span tiles or
    halves.
    """
    groups = []
    base = 0
    for t, (nlo, nhi) in enumerate(sched):
        ntile = nlo + nhi
        done = 0
        for half, n_c in ((0, nlo), (1, nhi)):
            off = 0
            while off < n_c:
                g = min(cpg, n_c - off)
                groups.append(dict(
                    tile=t, base=base + done + off, size=g, half=half,
                    first=(done + off == 0),
                    last=(done + off + g == ntile),
                ))
                off += g
            done += n_c
        base += ntile
        assert ntile > 0
    return groups


# ----------------------------------------------------------------------------
# Launch A: H = x @ W1_aug (row shard)
# ----------------------------------------------------------------------------
def build_launch_a(nper, ntiles_core, k_in, n_w):
    nc = bacc.Bacc("TRN2", target_bir_lowering=False, debug=False)
    xT = nc.dram_tensor("xT", (k_in, nper), F32, kind="ExternalInput")
    w = nc.dram_tensor("w1aug", (k_in, n_w), F32, kind="ExternalInput")
    h_out = nc.dram_tensor("h_out", (nper, n_w), F32, kind="ExternalOutput")
    kt = k_in // P
    nchunk_sizes = []
    off = 0
    while off < n_w:
        nchunk_sizes.append(min(512, n_w - off))
        off += nchunk_sizes[-1]
    with tile.TileContext(nc) as tc:
        with tc.tile_pool(name="sb", bufs=3) as sb, \
             tc.tile_pool(name="cst", bufs=1) as cst, \
             tc.tile_pool(name="ps", bufs=4, space="PSUM") as ps:
            w_sb = cst.tile([P, kt, n_w], F32)
            for k in range(kt):
                nc.sync.dma_start(w_sb[:, k, :], w[ds(k * P, P), :])
            for t in range(ntiles_core):
                rows = min(P, nper - t * P)
                xt_sb = sb.tile([P, kt, P], F32, tag="xt", name="xt_sb")
                for k in range(kt):
                    nc.sync.dma_start(xt_sb[:, k, 0:rows],
                                      xT[ds(k * P, P), ds(t * P, rows)])
                stage = sb.tile([P, n_w], F32, tag="hst", name="stage")
                noff = 0
                for nsz in nchunk_sizes:
                    acc = ps.tile([P, 512], F32, space="PSUM", tag="acc", name="acc")
                    for k in range(kt):
                        nc.tensor.matmul(acc[0:rows, 0:nsz], xt_sb[:, k, 0:rows],
                                         w_sb[:, k, ds(noff, nsz)],
                                         start=(k == 0), stop=(k == kt - 1))
                    nc.vector.tensor_copy(stage[0:rows, ds(noff, nsz)],
                                          acc[0:rows, 0:nsz])
                    noff += nsz
                nc.sync.dma_start(h_out[ds(t * P, rows), :], stage[0:rows, :])
    nc.compile()
    return nc


# ----------------------------------------------------------------------------
# Launch B: layer-1 aggregation + z23_aug
# ----------------------------------------------------------------------------
def build_launch_b(g1, nper, ntiles_core, half, rw, heads, chid, czw,
                   emb_w, zaug_w, cpg):
    """g1: host shard dict for graph 1 (this builds a per-core-specialized
    program -- chunk schedules differ per core, so we build one nc per core?
    No: SPMD needs ONE program. The schedule must be IDENTICAL across cores.
    Caller pads schedules to a common shape."""
    nc = bacc.Bacc("TRN2", target_bir_lowering=False, debug=False)
    tlo = nc.dram_tensor("tlo", (half, rw), F32, kind="ExternalInput")
    thi = nc.dram_tensor("thi", (half, rw), F32, kind="ExternalInput")
    adst_t = nc.dram_tensor("adst_t", (nper + 16, 64), F32, kind="ExternalInput")
    idxw = nc.dram_tensor("idxw", (P, g1["w16"]), I16, kind="ExternalInput")
    adstw_t = nc.dram_tensor("adstw", (P, g1["w16"]), I16, kind="ExternalInput")
    dstlocw = nc.dram_tensor("dstlocw", (P, g1["nchunks"]), F32, kind="ExternalInput")
    iota_row = nc.dram_tensor("iota_row", (P, P), F32, kind="ExternalInput")
    kvec = nc.dram_tensor("kvec", (P, heads), F32, kind="ExternalInput")
    b1b = nc.dram_tensor("b1b", (P, emb_w), F32, kind="ExternalInput")
    w23 = nc.dram_tensor("w23", (emb_w, zaug_w), F32, kind="ExternalInput")
    ident = nc.dram_tensor("ident", (P, P), F32, kind="ExternalInput")
    z_out = nc.dram_tensor("z_out", (nper, zaug_w), F32, kind="ExternalOutput")
    emb_dram = nc.dram_tensor("emb_scratch", (ntiles_core * P, emb_w), F32,
                              kind="Internal")

    groups = make_gather_groups(g1["sched"], cpg)
    with tile.TileContext(nc) as tc:
        with tc.tile_pool(name="sb", bufs=2) as sb, \
             tc.tile_pool(name="cst", bufs=1) as cst:
          with tc.tile_pool(name="ps_agg", bufs=1, space="PSUM") as ps:
            iota_sb = cst.tile([P, P], F32)
            nc.sync.dma_start(iota_sb[:], iota_row[:])
            k_sb = cst.tile([P, heads], F32)
            nc.sync.dma_start(k_sb[:], kvec[:])
            idx_sb = cst.tile([P, g1["w16"]], I16)
            nc.sync.dma_start(idx_sb[:], idxw[:])
            adidx_sb = cst.tile([P, g1["w16"]], I16)
            nc.sync.dma_start(adidx_sb[:], adstw_t[:])
            dl_sb = cst.tile([P, g1["nchunks"]], F32)
            nc.sync.dma_start(dl_sb[:], dstlocw[:])
            b1_sb = cst.tile([P, emb_w], F32)
            nc.sync.dma_start(b1_sb[:], b1b[:])

            accs = []
            for h in range(heads):
                acc_h = ps.tile([P, chid + 1], F32, space="PSUM",
                                tag=f"acc{h}", name=f"acc{h}")
                accs.append(acc_h)

            cur_tile = [None]

            def finalize_tile(t):
                rows = min(P, nper - t * P)
                emb_sb = sb.tile([P, emb_w], F32, tag="emb", name="emb_sb")
                for h in range(heads):
                    den = sb.tile([P, 1], F32, tag="den", name="den")
                    nc.vector.tensor_scalar_add(den[:], accs[h][:, chid:chid + 1],
                                                1e-30)
                    rec = sb.tile([P, 1], F32, tag="rec", name="rec")
                    nc.vector.reciprocal(rec[:], den[:])
                    # v = num*rec + b1 ; emb = elu(v)
                    v = sb.tile([P, chid], F32, tag="vv", name="v")
                    nc.vector.scalar_tensor_tensor(
                        out=v[:], in0=accs[h][:, 0:chid], scalar=rec[:],
                        in1=b1_sb[:, ds(h * chid, chid)],
                        op0=ALU.mult, op1=ALU.add)
                    vneg = sb.tile([P, chid], F32, tag="vneg", name="vneg")
                    nc.vector.tensor_scalar_min(vneg[:], v[:], 0.0)
                    evn = sb.tile([P, chid], F32, tag="evn", name="evn")
                    nc.scalar.activation(evn[:], vneg[:], AF.Exp)
                    vpos = sb.tile([P, chid], F32, tag="vpos", name="vpos")
                    nc.vector.tensor_scalar_max(vpos[:], v[:], 0.0)
                    nc.vector.scalar_tensor_tensor(
                        out=emb_sb[:, ds(h * chid, chid)], in0=evn[:],
                        scalar=-1.0, in1=vpos[:], op0=ALU.add, op1=ALU.add)
                nc.sync.dma_start(emb_dram[ds(t * P, rows), :], emb_sb[0:rows, :])

            for g in groups:
                gsz = g["size"]
                table = tlo if g["half"] == 0 else thi
                g_sb = sb.tile([P, cpg, rw], F32, tag="gath", name="g_sb")
                nc.gpsimd.dma_gather(
                    out_ap=g_sb[:, 0:gsz, :], in_ap=table[:],
                    idxs_ap=idx_sb[:, ds(g["base"] * 8, gsz * 8)],
                    num_idxs=gsz * P, num_idxs_reg=gsz * P, elem_size=rw)
                ga_sb = sb.tile([P, cpg, 64], F32, tag="gadst", name="ga_sb")
                nc.gpsimd.dma_gather(
                    out_ap=ga_sb[:, 0:gsz, :], in_ap=adst_t[:],
                    idxs_ap=adidx_sb[:, ds(g["base"] * 8, gsz * 8)],
                    num_idxs=gsz * P, num_idxs_reg=gsz * P, elem_size=64)
                # e = lrelu(asrc + adst); ex = exp(e - K)
                e_sb = sb.tile([P, cpg, heads], F32, tag="ee", name="e_sb")
                nc.vector.tensor_tensor(
                    e_sb[:, 0:gsz, :],
                    g_sb[:, 0:gsz, ds(heads * (chid + 1), heads)],
                    ga_sb[:, 0:gsz, 0:heads], op=ALU.add)
                es_sb = sb.tile([P, cpg, heads], F32, tag="es", name="es_sb")
                nc.vector.tensor_scalar_mul(es_sb[:, 0:gsz, :], e_sb[:, 0:gsz, :],
                                            NEG_SLOPE)
                nc.vector.tensor_tensor(e_sb[:, 0:gsz, :], e_sb[:, 0:gsz, :],
                                        es_sb[:, 0:gsz, :], op=ALU.max)
                nc.vector.tensor_tensor(
                    e_sb[:, 0:gsz, :], e_sb[:, 0:gsz, :],
                    k_sb[:, None, :].to_broadcast([P, gsz, heads]),
                    op=ALU.subtract)
                ex_sb = sb.tile([P, cpg, heads], F32, tag="ex", name="ex_sb")
                nc.scalar.activation(ex_sb[:, 0:gsz, :], e_sb[:, 0:gsz, :], AF.Exp)
                for c in range(gsz):
                    ch = g["base"] + c
                    s_all = sb.tile([P, heads, P], F32, tag="sall", name="s_all")
                    nc.vector.scalar_tensor_tensor(
                        out=s_all[:],
                        in0=iota_sb[:, None, :].to_broadcast([P, heads, P]),
                        scalar=dl_sb[:, ds(ch, 1)],
                        in1=ex_sb[:, c, :, None].to_broadcast([P, heads, P]),
                        op0=ALU.is_equal, op1=ALU.mult)
                    first = g["first"] and c == 0
                    last = g["last"] and c == gsz - 1
                    for h in range(heads):
                        nc.tensor.matmul(
                            accs[h][:], s_all[:, h, :],
                            g_sb[:, c, ds(h * (chid + 1), chid + 1)],
                            start=first, stop=last)
                if g["last"]:
                    finalize_tile(g["tile"])
                    cur_tile[0] = g["tile"]

          # ---- z phase: z23 = emb @ w23 (transpose emb tiles via PE)
          with tc.tile_pool(name="ps_z", bufs=2, space="PSUM") as ps:
            w23_sb = cst.tile([P, emb_w // P, zaug_w], F32)
            for k in range(emb_w // P):
                nc.sync.dma_start(w23_sb[:, k, :], w23[ds(k * P, P), :])
            id_sb = cst.tile([P, P], F32)
            nc.sync.dma_start(id_sb[:], ident[:])
            for t in range(ntiles_core):
                rows = min(P, nper - t * P)
                emb_in = sb.tile([P, emb_w], F32, tag="embr", name="emb_in")
                nc.vector.memset(emb_in[:], 0.0)
                nc.sync.dma_start(emb_in[0:rows, :], emb_dram[ds(t * P, rows), :])
                embT = sb.tile([P, emb_w // P, P], F32, tag="embT", name="embT")
                for k in range(emb_w // P):
                    tp = ps.tile([P, P], F32, space="PSUM", tag="tp", name="tp")
                    nc.tensor.transpose(tp[:], emb_in[:, ds(k * P, P)], id_sb[:])
                    nc.vector.tensor_copy(embT[:, k, :], tp[:])
                zacc = ps.tile([P, zaug_w], F32, space="PSUM", tag="zacc", name="zacc")
                for k in range(emb_w // P):
                    nc.tensor.matmul(zacc[0:rows, :], embT[:, k, 0:rows],
                                     w23_sb[:, k, :],
                                     start=(k == 0), stop=(k == emb_w // P - 1))
                zst = sb.tile([P, zaug_w], F32, tag="zst", name="zst")
                nc.vector.tensor_copy(zst[0:rows, :], zacc[0:rows, :])
                nc.sync.dma_start(z_out[ds(t * P, rows), :], zst[0:rows, :])
    nc.compile()
    return nc


# ----------------------------------------------------------------------------
# Launch C: layers 2/3 aggregation + softmax + argmax
# ----------------------------------------------------------------------------
def build_launch_c(shards, nper, ntiles_core, half, zrw, outw, cpg):
    """shards: [g1, g2] host dicts (graph1 -> logits, graph2 -> logits_2)."""
    nc = bacc.Bacc("TRN2", target_bir_lowering=False, debug=False)
    tabs = []
    for gi in (1, 2):
        tabs.append((
            nc.dram_tensor(f"z{gi}lo", (half, zrw), F32, kind="ExternalInput"),
            nc.dram_tensor(f"z{gi}hi", (half, zrw), F32, kind="ExternalInput"),
            nc.dram_tensor(f"a{gi}dst", (nper + 16, 64), F32, kind="ExternalInput"),
        ))
    ins = []
    for gi, g in ((1, shards[0]), (2, shards[1])):
        ins.append((
            nc.dram_tensor(f"idxw{gi}", (P, g["w16"]), I16, kind="ExternalInput"),
            nc.dram_tensor(f"adstw{gi}", (P, g["w16"]), I16, kind="ExternalInput"),
            nc.dram_tensor(f"dstlocw{gi}", (P, g["nchunks"]), F32, kind="ExternalInput"),
        ))
    iota_row = nc.dram_tensor("iota_row", (P, P), F32, kind="ExternalInput")
    riota = nc.dram_tensor("riota", (P, P), F32, kind="ExternalInput")
    kv2 = nc.dram_tensor("kv2", (P, 2), F32, kind="ExternalInput")
    b2b = nc.dram_tensor("b2b", (P, 2 * outw), F32, kind="ExternalInput")
    louts = [
        nc.dram_tensor("logits1", (nper, outw), F32, kind="ExternalOutput"),
        nc.dram_tensor("logits2", (nper, outw), F32, kind="ExternalOutput"),
    ]
    pred_out = nc.dram_tensor("preds", (nper, 1), I32, kind="ExternalOutput")

    with tile.TileContext(nc) as tc:
        with tc.tile_pool(name="sb", bufs=2) as sb, \
             tc.tile_pool(name="cst", bufs=1) as cst, \
             tc.tile_pool(name="ps", bufs=2, space="PSUM") as ps:
            iota_sb = cst.tile([P, P], F32)
            nc.sync.dma_start(iota_sb[:], iota_row[:])
            ri_sb = cst.tile([P, P], F32)
            nc.sync.dma_start(ri_sb[:], riota[:])
            kv_sb = cst.tile([P, 2], F32)
            nc.sync.dma_start(kv_sb[:], kv2[:])
            b2_sb = cst.tile([P, 2 * outw], F32)
            nc.sync.dma_start(b2_sb[:], b2b[:])

            for gi in (0, 1):
                g = shards[gi]
                zlo, zhi, adst_t = tabs[gi]
                idxw, adstw_t, dstlocw = ins[gi]
                idx_sb = sb.tile([P, g["w16"]], I16, tag="idxg", name="idx_sb")
                nc.sync.dma_start(idx_sb[:], idxw[:])
                adidx_sb = sb.tile([P, g["w16"]], I16, tag="adidxg", name="adidx_sb")
                nc.sync.dma_start(adidx_sb[:], adstw_t[:])
                dl_sb = sb.tile([P, g["nchunks"]], F32, tag="dlg", name="dl_sb")
                nc.sync.dma_start(dl_sb[:], dstlocw[:])
                groups = make_gather_groups(g["sched"], cpg)
                for grp in groups:
                    gsz = grp["size"]
                    table = zlo if grp["half"] == 0 else zhi
                    g_sb = sb.tile([P, cpg, zrw], F32, tag="gath", name="g_sb")
                    nc.gpsimd.dma_gather(
                        out_ap=g_sb[:, 0:gsz, :], in_ap=table[:],
                        idxs_ap=idx_sb[:, ds(grp["base"] * 8, gsz * 8)],
                        num_idxs=gsz * P, num_idxs_reg=gsz * P, elem_size=zrw)
                    ga_sb = sb.tile([P, cpg, 64], F32, tag="gadst", name="ga_sb")
                    nc.gpsimd.dma_gather(
                        out_ap=ga_sb[:, 0:gsz, :], in_ap=adst_t[:],
                        idxs_ap=adidx_sb[:, ds(grp["base"] * 8, gsz * 8)],
                        num_idxs=gsz * P, num_idxs_reg=gsz * P, elem_size=64)
                    e_sb = sb.tile([P, cpg, 1], F32, tag="ee", name="e_sb", bufs=2)
                    nc.vector.tensor_tensor(
                        e_sb[:, 0:gsz, :], g_sb[:, 0:gsz, ds(outw + 1, 1)],
                        ga_sb[:, 0:gsz, 0:1], op=ALU.add)
                    es_sb = sb.tile([P, cpg, 1], F32, tag="es", name="es_sb", bufs=2)
                    nc.vector.tensor_scalar_mul(es_sb[:, 0:gsz, :],
                                                e_sb[:, 0:gsz, :], NEG_SLOPE)
                    nc.vector.tensor_tensor(e_sb[:, 0:gsz, :], e_sb[:, 0:gsz, :],
                                            es_sb[:, 0:gsz, :], op=ALU.max)
                    nc.vector.tensor_tensor(
                        e_sb[:, 0:gsz, :], e_sb[:, 0:gsz, :],
                        kv_sb[:, None, ds(gi, 1)].to_broadcast([P, gsz, 1]),
                        op=ALU.subtract)
                    ex_sb = sb.tile([P, cpg, 1], F32, tag="ex", name="ex_sb", bufs=2)
                    nc.scalar.activation(ex_sb[:, 0:gsz, :], e_sb[:, 0:gsz, :],
                                         AF.Exp)
                    if grp["first"]:
                        acc = ps.tile([P, outw + 1], F32, space="PSUM",
                                      tag="acc", name="acc")
                        cur_acc = acc
                    else:
                        acc = cur_acc
                    for c in range(gsz):
                        ch = grp["base"] + c
                        s_sb = sb.tile([P, P], F32, tag="sone", name="s_sb", bufs=2)
                        nc.vector.scalar_tensor_tensor(
                            out=s_sb[:], in0=iota_sb[:],
                            scalar=dl_sb[:, ds(ch, 1)],
                            in1=ex_sb[:, c, 0:1].to_broadcast([P, P]),
                            op0=ALU.is_equal, op1=ALU.mult)
                        nc.tensor.matmul(
                            acc[:], s_sb[:], g_sb[:, c, 0:outw + 1],
                            start=(grp["first"] and c == 0),
                            stop=(grp["last"] and c == gsz - 1))
                    if grp["last"]:
                        t = grp["tile"]
                        rows = min(P, nper - t * P)
                        den = sb.tile([P, 1], F32, tag="den", name="den")
                        nc.vector.tensor_scalar_add(den[:], acc[:, outw:outw + 1],
                                                    1e-30)
                        rec = sb.tile([P, 1], F32, tag="rec", name="rec")
                        nc.vector.reciprocal(rec[:], den[:])
                        xv = sb.tile([P, outw], F32, tag="xv", name="xv")
                        nc.vector.scalar_tensor_tensor(
                            out=xv[:], in0=acc[:, 0:outw], scalar=rec[:],
                            in1=b2_sb[:, ds(gi * outw, outw)],
                            op0=ALU.mult, op1=ALU.add)
                        # softmax over features
                        mx = sb.tile([P, 1], F32, tag="mx", name="mx")
                        nc.vector.tensor_reduce(mx[:], xv[:],
                                                axis=mybir.AxisListType.X,
                                                op=ALU.max)
                        nmx = sb.tile([P, 1], F32, tag="nmx", name="nmx")
                        nc.vector.tensor_scalar_mul(nmx[:], mx[:], -1.0)
                        emv = sb.tile([P, outw], F32, tag="emv", name="emv")
                        ssum = sb.tile([P, 1], F32, tag="ssum", name="ssum")
                        nc.scalar.activation(emv[:], xv[:], AF.Exp, bias=nmx[:],
                                             accum_out=ssum[:])
                        sr = sb.tile([P, 1], F32, tag="sr", name="sr")
                        nc.vector.reciprocal(sr[:], ssum[:])
                        lg = sb.tile([P, outw], F32, tag="lg", name="lg")
                        nc.vector.tensor_scalar_mul(lg[:], emv[:], sr[:])
                        nc.sync.dma_start(louts[gi][ds(t * P, rows), :],
                                          lg[0:rows, :])
                        if gi == 0:
                            # argmax: first index attaining max
                            am = sb.tile([P, P], F32, tag="am", name="am")
                            nc.vector.scalar_tensor_tensor(
                                out=am[:], in0=xv[:], scalar=mx[:],
                                in1=ri_sb[:], op0=ALU.is_equal, op1=ALU.mult)
                            rmx = sb.tile([P, 1], F32, tag="rmx", name="rmx")
                            nc.vector.tensor_reduce(rmx[:], am[:],
                                                    axis=mybir.AxisListType.X,
                                                    op=ALU.max)
                            pr = sb.tile([P, 1], F32, tag="pr", name="pr")
                            nc.vector.tensor_scalar(
                                out=pr[:], in0=rmx[:], scalar1=-1.0,
                                scalar2=float(outw - 1), op0=ALU.mult,
                                op1=ALU.add)
                            pri = sb.tile([P, 1], I32, tag="pri", name="pri")
                            nc.vector.tensor_copy(pri[:], pr[:])
                            nc.sync.dma_start(pred_out[ds(t * P, rows), :],
                                              pri[0:rows, :])
    nc.compile()
    return nc


# ----------------------------------------------------------------------------
# Schedule padding: SPMD needs identical programs across cores
# ----------------------------------------------------------------------------
def unify_shards(shards, nper, ntiles_core):
    """Pad per-core shard schedules to the max (nlo, nhi) per tile across
    cores so one program fits all. Pad chunks reference src row 0 / adst pad
    row, contributing exp(-huge)=0."""
    ncores = len(shards)
    maxsched = []
    for t in range(ntiles_core):
        mlo = max(s["sched"][t][0] for s in shards)
        mhi = max(s["sched"][t][1] for s in shards)
        maxsched.append((mlo, mhi))
    out = []
    for s in shards:
        src_parts, adst_parts, dl_parts = [], [], []
        # reconstruct per-tile segments from flat arrays
        srcw = s["srcw"][:16]  # [16, w16]
        src_flat = srcw.T.reshape(-1)
        adst_flat = s["adstw"][:16].T.reshape(-1)
        dl_flat = s["dstlocw"].T.reshape(-1)
        base = 0
        for t in range(ntiles_core):
            nlo, nhi = s["sched"][t]
            mlo, mhi = maxsched[t]
            for n_c, m_c in ((nlo, mlo), (nhi, mhi)):
                seg = slice(base * P, (base + n_c) * P)
                src_parts.append(src_flat[seg])
                adst_parts.append(adst_flat[seg])
                dl_parts.append(dl_flat[seg])
                padc = m_c - n_c
                if padc:
                    src_parts.append(np.zeros(padc * P, src_flat.dtype))
                    adst_parts.append(np.full(padc * P, nper, adst_flat.dtype))
                    dl_parts.append(np.zeros(padc * P, dl_flat.dtype))
                base += n_c
        src_idx = np.concatenate(src_parts)
        adst_idx = np.concatenate(adst_parts)
        dst_loc = np.concatenate(dl_parts).astype(np.float32)
        epad = src_idx.shape[0]
        out.append(dict(
            srcw=np.tile(src_idx.reshape(-1, 16).T, (8, 1)).astype(np.int16),
            adstw=np.tile(adst_idx.reshape(-1, 16).T, (8, 1)).astype(np.int16),
            dstlocw=dst_loc.reshape(-1, P).T.copy(),
            sched=maxsched, nchunks=epad // P, w16=epad // 16,
        ))
    return out


# ----------------------------------------------------------------------------
# Main kernel
# ----------------------------------------------------------------------------
def kernel(x, edge_index, edge_index_2, W1, att_src1, att_dst1, b1,
           W2, att_src2, att_dst2, b2, W3, att_src3, att_dst3, b3):
    t_start = time.time()
    x = np.asarray(x, np.float32)
    n, k_in = x.shape
    heads, chid = att_src1.shape
    outw = W2.shape[1]
    emb_w = heads * chid
    nper = n // NCORES
    ntiles_core = -(-nper // P)
    half = n // 2
    rw = heads * (chid + 1) + 2 * heads
    rw = -(-rw * 4 // 256) * 64  # round row bytes to 256B, in fp32 elems
    zrw = -(-(outw + 3) * 4 // 256) * 64
    cpg = 2
    cpg_c = 6

    W1 = np.asarray(W1, np.float32)
    # ---- host prep: graphs
    def with_loops(ei):
        ei = np.asarray(ei, np.int64)
        loops = np.arange(n, dtype=np.int64)
        src = np.concatenate([ei[0], loops])
        dst = np.concatenate([ei[1], loops])
        return src, dst

    s1, d1 = with_loops(edge_index)
    s2, d2 = with_loops(edge_index_2)
    shards1 = [prep_graph_shard(s1, d1, n, c, nper, half, ntiles_core)
               for c in range(NCORES)]
    shards2 = [prep_graph_shard(s2, d2, n, c, nper, half, ntiles_core)
               for c in range(NCORES)]
    u1 = unify_shards(shards1, nper, ntiles_core)
    u2 = unify_shards(shards2, nper, ntiles_core)
    _log(f"host prep {time.time()-t_start:.1f}s; chunks/core g1={u1[0]['nchunks']}"
         f" g2={u2[0]['nchunks']}")

    # ---- weight prep
    W1h = W1.reshape(k_in, heads, chid)
    a_src_cols = np.einsum("khc,hc->kh", W1h, np.asarray(att_src1, np.float32))
    a_dst_cols = np.einsum("khc,hc->kh", W1h, np.asarray(att_dst1, np.float32))
    w1aug = np.concatenate([W1, a_src_cols, a_dst_cols], 1)  # [k_in, emb_w+16]
    n_w = w1aug.shape[1]

    # ---- launch A
    t0 = time.time()
    ncA = build_launch_a(nper, ntiles_core, k_in, n_w)
    runA = make_runner(ncA, shared_names={"w1aug"})
    _log(f"launch A built {time.time()-t0:.1f}s")
    xT = np.ascontiguousarray(x.T)
    perA = [dict(xT=np.ascontiguousarray(xT[:, c * nper:(c + 1) * nper]))
            for c in range(NCORES)]
    runA.put_inputs(dict(w1aug=w1aug), perA)
    t0 = time.time()
    outsA = runA.run()
    _log(f"launch A ran {time.time()-t0:.1f}s")
    resA = runA.results(outsA)
    H = np.concatenate([r["h_out"] for r in resA], 0)  # [n, emb_w+16]
    tA = min(runA.time_runs(2))

    # ---- assemble layer-1 gather table
    asrc = H[:, emb_w:emb_w + heads]
    adst = H[:, emb_w + heads:emb_w + 2 * heads]
    K1 = np.float32(asrc.max(0) + adst.max(0))  # upper bound on e per head
    table1 = np.zeros((n, rw), np.float32)
    hc = H[:, :emb_w].reshape(n, heads, chid)
    t1v = table1[:, :heads * (chid + 1)].reshape(n, heads, chid + 1)
    t1v[:, :, :chid] = hc
    t1v[:, :, chid] = 1.0
    table1[:, heads * (chid + 1):heads * (chid + 1) + heads] = asrc
    adst_tabs = []
    for c in range(NCORES):
        at = np.full((nper + 16, 64), -1e9, np.float32)
        at[:nper, :heads] = adst[c * nper:(c + 1) * nper]
        adst_tabs.append(at)
    del H, hc

    # ---- launch B
    W2 = np.asarray(W2, np.float32)
    W3 = np.asarray(W3, np.float32)
    a2s = W2 @ np.asarray(att_src2, np.float32).reshape(-1)
    a2d = W2 @ np.asarray(att_dst2, np.float32).reshape(-1)
    a3s = W3 @ np.asarray(att_src3, np.float32).reshape(-1)
    a3d = W3 @ np.asarray(att_dst3, np.float32).reshape(-1)
    zaug_w = 2 * (outw + 2)
    w23 = np.concatenate([W2, a2s[:, None], a2d[:, None],
                          W3, a3s[:, None], a3d[:, None]], 1)
    b1b = np.tile(np.asarray(b1, np.float32), (P, 1))
    iota = np.tile(np.arange(P, dtype=np.float32), (P, 1))
    ident = np.eye(P, dtype=np.float32)
    kvecb = np.tile(K1, (P, 1))

    t0 = time.time()
    ncB = build_launch_b(u1[0], nper, ntiles_core, half, rw, heads, chid, zrw,
                         emb_w, zaug_w, cpg)
    runB = make_runner(ncB, shared_names={"tlo", "thi", "iota_row", "kvec",
                                         "b1b", "w23", "ident"})
    _log(f"launch B built {time.time()-t0:.1f}s "
         f"({len(ncB.m.functions[0].blocks[0].instructions) if ncB.m.functions[0].blocks else '?'} insts)")
    sharedB = dict(tlo=table1[:half], thi=table1[half:], iota_row=iota,
                   kvec=kvecb, b1b=b1b, w23=w23, ident=ident)
    perB = [dict(adst_t=adst_tabs[c], idxw=u1[c]["srcw"], adstw=u1[c]["adstw"],
                 dstlocw=u1[c]["dstlocw"]) for c in range(NCORES)]
    runB.put_inputs(sharedB, perB)
    t0 = time.time()
    outsB = runB.run()
    _log(f"launch B ran {time.time()-t0:.1f}s")
    resB = runB.results(outsB)
    Z = np.concatenate([r["z_out"] for r in resB], 0)  # [n, zaug_w]
    tB = min(runB.time_runs(2))
    del table1, sharedB

    # ---- assemble z tables
    z2 = Z[:, 0:outw]
    a2srcv = Z[:, outw]
    a2dstv = Z[:, outw + 1]
    z3 = Z[:, outw + 2:2 * outw + 2]
    a3srcv = Z[:, 2 * outw + 2]
    a3dstv = Z[:, 2 * outw + 3]
    K2 = np.float32(a2srcv.max() + a2dstv.max())
    K3 = np.float32(a3srcv.max() + a3dstv.max())

    def ztable(z, asr):
        tab = np.zeros((n, zrw), np.float32)
        tab[:, 0:outw] = z
        tab[:, outw] = 1.0
        tab[:, outw + 1] = asr
        return tab

    ztab2 = ztable(z2, a2srcv)
    ztab3 = ztable(z3, a3srcv)

    def adtab(adv):
        outl = []
        for c in range(NCORES):
            at = np.full((nper + 16, 64), -1e9, np.float32)
            at[:nper, 0] = adv[c * nper:(c + 1) * nper]
            outl.append(at)
        return outl

    ad2 = adtab(a2dstv)
    ad3 = adtab(a3dstv)
    riota = np.tile(np.arange(P - 1, -1, -1, dtype=np.float32), (P, 1))
    b2b = np.tile(np.concatenate([np.asarray(b2, np.float32),
                                  np.asarray(b3, np.float32)]), (P, 1))
    kv2 = np.tile(np.array([K2, K3], np.float32), (P, 1))

    t0 = time.time()
    ncC = build_launch_c([u1[0], u2[0]], nper, ntiles_core, half, zrw, outw,
                         cpg_c)
    runC = make_runner(ncC, shared_names={"z1lo", "z1hi", "z2lo", "z2hi",
                                         "iota_row", "riota", "kv2", "b2b"})
    _log(f"launch C built {time.time()-t0:.1f}s")
    sharedC = dict(z1lo=ztab2[:half], z1hi=ztab2[half:],
                   z2lo=ztab3[:half], z2hi=ztab3[half:],
                   iota_row=iota, riota=riota, kv2=kv2, b2b=b2b)
    perC = [dict(a1dst=ad2[c], a2dst=ad3[c],
                 idxw1=u1[c]["srcw"], adstw1=u1[c]["adstw"],
                 dstlocw1=u1[c]["dstlocw"],
                 idxw2=u2[c]["srcw"], adstw2=u2[c]["adstw"],
                 dstlocw2=u2[c]["dstlocw"]) for c in range(NCORES)]
    runC.put_inputs(sharedC, perC)
    t0 = time.time()
    outsC = runC.run()
    _log(f"launch C ran {time.time()-t0:.1f}s")
    resC = runC.results(outsC)
    tC = min(runC.time_runs(2))

    logits = np.concatenate([r["logits1"] for r in resC], 0)
    logits_2 = np.concatenate([r["logits2"] for r in resC], 0)
    predictions = np.concatenate([r["preds"] for r in resC], 0)[:, 0].astype(np.int32)
    _log(f"total kernel() wall {time.time()-t_start:.1f}s; warm walls "
         f"A={tA*1e3:.1f}ms B={tB*1e3:.1f}ms C={tC*1e3:.1f}ms")
    kernel.last_times = (tA, tB, tC)
    return logits, logits_2, predictions
